# revision 31
# baseline (speedup 1.0000x reference)
"""MetaNetImageEncoder Trainium2 kernel.

Data-parallel over batch: 8 samples per NeuronCore x 8 cores.

Per core (sample-local b in 0..7, D=768, N=196 patches, T=8 tasks):
  1. base pass:   A = P @ W1 in fp8e4m3 (W1 staged at 64x scale),
                  pooled = mean_n relu(A + b1); relu+pool alternates
                  between ScalarE (activation+accum) and VectorE
                  (tensor_scalar add/max + accum). base2 = W2.T@pooled
                  matmuls are interleaved into the phase-1 k-loop.
  2. MetaNet:     coefs[t,b] via two small matmul chains.
  3. mixing:      64*M_b = (4c)@(16*dW1) via a block-diagonal fp8
                  stationary; PSUM partitions = (sample, i%16); copies
                  cast to fp8 mxiall (64x scale).
  4. final pass:  one merged gather-DMA per sample de-interleaves
                  64*M_b; DVE adds 64*W1 (same fp8 tensor as phase 1);
                  H = relu(P @ 64*nW1 + 64*nb1), pooled_new at 64x.
  5. layer 2:     out = pooled@W2 + sum_t c (pooled@dW2[t]) + b2 + c@db2;
                  dW2 staged fp8 at 16x, un-scaled in the PSUM->SBUF copy.

All fp8 scale factors cancel: biases are staged at 64x, pooling
normalization divides by 196*64.
"""
import numpy as np
import ml_dtypes

import concourse.bass as bass
import concourse.mybir as mybir
import concourse.tile as tile
from concourse.vector_clock import ScopedClock
from concourse.bass_utils import run_bass_kernel_spmd

F32 = mybir.dt.float32
BF16 = mybir.dt.bfloat16
E4 = mybir.dt.float8e4
RELU = mybir.ActivationFunctionType.Relu
ADD = mybir.AluOpType.add
MAX = mybir.AluOpType.max

P = 16
D = 768
T = 8
HM = 192
NPAT = 196          # 14*14 patches
B = 64
NCORES = 8
BC = B // NCORES    # 8 samples per core
NB = BC * NPAT      # 1568
KT = D // 128       # 6 k-tiles

SW = 64.0           # W1 fp8 staging scale (phase-1/4 psum is 64x)
SD = 16.0           # dW1/dW2 fp8 staging scale
SCC = SW / SD       # coef scale inside cb (4.0) so cb@dw1 = 64*M
POOL_SCALE = 1.0 / (NPAT * SW)

_PATCHED = False


def _apply_tile_patch():
    """This container's walrus allows only one sem wait per instruction;
    TileContext's exit drain attaches one wait per live semaphore. Split
    them onto standalone single-wait nops."""
    global _PATCHED
    if _PATCHED:
        return
    _PATCHED = True

    def _patched(self, tick_clock, wait_clock):
        carrier = self.nc.sync.nop(nofuse=True, hint="drain_waits")
        wait_clock.add_sem_waits(
            carrier.ins, ScopedClock({None: tick_clock.global_clock})
        )
        si = carrier.ins.sync_info
        waits = list(si.on_wait) if si else []
        if len(waits) > 1:
            carrier.ins.sync_info = mybir.SyncInfo(on_wait=[waits[0]], on_update=[])
            for w in waits[1:]:
                extra = self.nc.sync.nop(nofuse=True, hint="drain_waits")
                extra.ins.sync_info = mybir.SyncInfo(on_wait=[w], on_update=[])
        self.nc.sync.drain()
        self.nc.all_engine_barrier()
        popped = self.nc._tile_sem_poison_stack.pop()
        assert popped is self._sem_poison
        self.nc.clear_and_free_semaphores(list(self.sems.allocated().values()))
        self.nc.all_engine_barrier()

    tile.TileContext._drain_and_barrier = _patched


def _split_multi_waits(nc, max_waits: int = 1):
    """Hoist extra sem waits onto same-engine InstNoOp carriers."""
    for f in nc.m.functions:
        for blk in f.blocks:
            out = []
            for inst in blk.instructions:
                si = inst.sync_info
                if si is not None and len(si.on_wait) > max_waits:
                    waits = list(si.on_wait)
                    for i, w in enumerate(waits[:-max_waits]):
                        out.append(mybir.InstNoOp(
                            name=f"{inst.name}-w{i}",
                            sync_info=mybir.SyncInfo(on_wait=[w], on_update=[]),
                            bass_nofuse=True,
                            engine=inst.engine,
                        ))
                    inst.sync_info = mybir.SyncInfo(
                        on_wait=waits[-max_waits:], on_update=list(si.on_update)
                    )
                out.append(inst)
            blk.instructions = out


def build_kernel():
    nc = bass.Bass(target_bir_lowering=False, trn_type="TRN2")

    din = {}
    def inp(name, shape, dt):
        din[name] = nc.dram_tensor(name, shape, dt, kind="ExternalInput")
        return din[name]

    xt = inp("xt", (128, KT, NB), E4)            # patches^T  [i_local, kt, (b,n)]
    w1 = inp("w1", (128, KT, D), E4)             # 64*W1 [i_local, it, j]
    w2 = inp("w2", (128, KT, D), BF16)           # W2 [j_local, jt, e]
    dw1 = inp("dw1", (48, 128, D), E4)           # 16*dW1 [icl*6+it, (t,s16), j]
    dw2 = inp("dw2", (T, KT, 128, D), E4)        # 16*dW2 [t, kt, j_local, e]
    db1 = inp("db1", (T, D), BF16)               # 64*db1
    db2 = inp("db2", (T, D), BF16)               # db2 (1x)
    b1t = inp("b1t", (128, KT), F32)             # 64*b1 [j_local, jt]
    b1tn = inp("b1tn", (128, KT), F32)           # -64*b1
    b1ts = inp("b1ts", (128, KT), F32)           # 196*64*b1
    b2t = inp("b2t", (128, KT), F32)             # b2 [e_local, et] (1x)
    b2r = inp("b2r", (BC, D), F32)               # b2 replicated over samples
    mw1 = inp("mw1", (128, KT, HM), BF16)
    mb1t = inp("mb1t", (128, 2), F32)
    mw2 = inp("mw2", (128, 2, T), BF16)          # [h_local, g, t], g=1 padded
    mb2t = inp("mb2t", (T, 1), F32)
    iexp = inp("iexp", (T, 128), F32)            # 4*repeat(eye(8),16,axis=1)
    i64 = inp("i64", (128, 128), BF16)           # 64*eye(128)
    mask16 = inp("mask16", (128, P), BF16)       # [p, s'] = (p%16==s')
    i8 = inp("i8", (T, T), F32)                  # eye(8)

    out = nc.dram_tensor("out", (BC, D), F32, kind="ExternalOutput")

    with tile.TileContext(nc) as tc:
        with (
            tc.tile_pool(name="big", bufs=1) as big,
            tc.tile_pool(name="sm", bufs=1) as sm,
            tc.tile_pool(name="ring", bufs=6) as ring,
            tc.tile_pool(name="mxcp", bufs=3) as mxcp,
            tc.tile_pool(name="scr", bufs=4) as scr,
        ):
            pst = tc.alloc_tile_pool(name="pst", bufs=2, space="PSUM")
            # ---------- persistent loads (order = queue order) ----------
            w1_sb = big.tile([128, KT, D], E4, tag="w1")
            nc.sync.dma_start(w1_sb[:, :, 0:128], w1[:, :, 0:128])
            xt_sb = big.tile([128, KT, NB], E4, tag="xt")
            for kt in range(KT):
                nc.sync.dma_start(xt_sb[:, kt, :], xt[:, kt, :])
            for jt in range(1, KT):
                nc.sync.dma_start(w1_sb[:, :, jt * 128:(jt + 1) * 128],
                                  w1[:, :, jt * 128:(jt + 1) * 128])
            w2_sb = big.tile([128, KT, D], BF16, tag="w2")
            nc.scalar.dma_start(w2_sb[:], w2[:])
            mxiall = big.tile([128, 8, KT, D], E4, tag="mxiall")
            a_bf = big.tile([128, KT, NB], BF16, tag="a_bf")
            i64_sb = sm.tile([128, 128], BF16, tag="i64")
            nc.scalar.dma_start(i64_sb[:], i64[:])

            # smalls go on the gpsimd queue: scalar must stay free for
            # phase-1 activations, sync streams xt/w1
            b1t_sb = sm.tile([128, KT], F32, tag="b1t")
            nc.gpsimd.dma_start(b1t_sb[:], b1t[:])
            b1tn_sb = sm.tile([128, KT], F32, tag="b1tn")
            nc.gpsimd.dma_start(b1tn_sb[:], b1tn[:])
            b1ts_sb = sm.tile([128, KT], F32, tag="b1ts")
            nc.gpsimd.dma_start(b1ts_sb[:], b1ts[:])
            b2t_sb = sm.tile([128, KT], F32, tag="b2t")
            nc.gpsimd.dma_start(b2t_sb[:], b2t[:])
            b2r_sb = sm.tile([BC, D], F32, tag="b2r")
            nc.gpsimd.dma_start(b2r_sb[:], b2r[:])
            mw1_sb = sm.tile([128, KT, HM], BF16, tag="mw1")
            nc.gpsimd.dma_start(mw1_sb[:], mw1[:])
            mb1t_sb = sm.tile([128, 2], F32, tag="mb1t")
            nc.gpsimd.dma_start(mb1t_sb[:], mb1t[:])
            mw2_sb = sm.tile([128, 2, T], BF16, tag="mw2")
            nc.gpsimd.dma_start(mw2_sb[:], mw2[:])
            mb2t_sb = sm.tile([T, 1], F32, tag="mb2t")
            nc.gpsimd.dma_start(mb2t_sb[:], mb2t[:])
            iexp_sb = sm.tile([T, 128], F32, tag="iexp")
            nc.gpsimd.dma_start(iexp_sb[:], iexp[:])
            mask16_sb = sm.tile([128, P], BF16, tag="mask16")
            nc.gpsimd.dma_start(mask16_sb[:], mask16[:])
            i8_sb = sm.tile([T, T], F32, tag="i8")
            nc.gpsimd.dma_start(i8_sb[:], i8[:])
            db1_sb = sm.tile([T, D], BF16, tag="db1")
            nc.gpsimd.dma_start(db1_sb[:], db1[:])
            db2_sb = sm.tile([T, D], BF16, tag="db2")
            nc.gpsimd.dma_start(db2_sb[:], db2[:])

            # pre-warm the ACT spline table so ACT_TABLE_LOAD doesn't
            # stall the first real activation mid-phase-1
            warm = scr.tile([1, 1], BF16, tag="warm")
            nc.scalar.activation(warm[:], mb1t_sb[0:1, 0:1], RELU)

            poolb = sm.tile([128, KT * BC], F32, tag="poolb")
            pooln = sm.tile([128, KT * BC], F32, tag="pooln")
            poolb_bf = sm.tile([128, KT * BC], BF16, tag="poolbbf")

            # ---------- phase 1: base pass (+ interleaved base2) ----------
            psA = tc.alloc_tile_pool(name="psA", bufs=4, space="PSUM")
            psB2 = tc.alloc_tile_pool(name="psB2", bufs=2, space="PSUM")
            b2acc = sm.tile([128, KT, BC], F32, tag="b2acc")
            for jt in range(KT):
                for ch in range(4):      # 4 chunks of 392 = 2 samples
                    pa = psA.tile([128, 392], F32, tag="a")
                    for kt in range(KT):
                        nc.tensor.matmul(
                            pa[:],
                            w1_sb[:, kt, jt * 128:(jt + 1) * 128],
                            xt_sb[:, kt, ch * 392:(ch + 1) * 392],
                            start=(kt == 0), stop=(kt == KT - 1))
                    # stash A = X@W1 (1x) for phase-4 re-injection
                    nc.vector.tensor_scalar_mul(
                        a_bf[:, jt, ch * 392:(ch + 1) * 392], pa[:],
                        1.0 / SW)
                    for bi in range(2):
                        b = ch * 2 + bi
                        ro = scr.tile([128, NPAT], BF16, tag="ro")
                        col = poolb[:, jt * BC + b:jt * BC + b + 1]
                        if b % 2 == 0:
                            nc.scalar.activation(
                                ro[:], pa[:, bi * NPAT:(bi + 1) * NPAT], RELU,
                                bias=b1t_sb[:, jt:jt + 1], accum_out=col)
                        else:
                            # relu(a+b) pooled: out = max(a,-b) (discarded),
                            # accum = sum(max(a,-b)) + 196*b
                            nc.vector.tensor_scalar(
                                ro[:], pa[:, bi * NPAT:(bi + 1) * NPAT],
                                b1tn_sb[:, jt:jt + 1], b1ts_sb[:, jt:jt + 1],
                                op0=MAX, op1=ADD, accum_out=col)
                # base2 partial for this k-block, accumulated in SBUF f32
                nc.scalar.mul(poolb_bf[:, jt * BC:(jt + 1) * BC],
                              poolb[:, jt * BC:(jt + 1) * BC], POOL_SCALE)
                pb2 = psB2.tile([128, KT, BC], F32, tag="b2p")
                for et in range(KT):
                    nc.tensor.matmul(
                        pb2[:, et, :], w2_sb[:, jt, et * 128:(et + 1) * 128],
                        poolb_bf[:, jt * BC:(jt + 1) * BC],
                        start=True, stop=True)
                if jt == 0:
                    nc.vector.tensor_copy(b2acc[:], pb2[:])
                else:
                    nc.vector.tensor_tensor(b2acc[:], b2acc[:], pb2[:], op=ADD)

            # base2^T[e, b] = W2.T @ pooled + b2  (input to MetaNet)
            base2_bf = sm.tile([128, KT * BC], BF16, tag="base2bf")
            for et in range(KT):
                nc.vector.tensor_scalar_add(
                    base2_bf[:, et * BC:(et + 1) * BC], b2acc[:, et, :],
                    b2t_sb[:, et:et + 1])

            # ---------- phase 2: MetaNet ----------
            mh0 = sm.tile([128, T], BF16, tag="mh0")
            mh1 = sm.tile([64, T], BF16, tag="mh1")
            for g, mh_g in ((0, mh0), (1, mh1)):
                cols = 128 if g == 0 else 64
                pm = pst.tile([cols, T], F32, tag="tiny")
                for kt in range(KT):
                    nc.tensor.matmul(
                        pm[:], mw1_sb[:, kt, g * 128:g * 128 + cols],
                        base2_bf[:, kt * BC:(kt + 1) * BC],
                        start=(kt == 0), stop=(kt == KT - 1))
                nc.scalar.activation(mh_g[:], pm[:], RELU,
                                     bias=mb1t_sb[:cols, g:g + 1])

            pc = pst.tile([T, T], F32, tag="tiny")
            nc.tensor.matmul(pc[:], mw2_sb[:, 0, :], mh0[:], start=True, stop=False)
            nc.tensor.matmul(pc[:], mw2_sb[0:64, 1, :], mh1[:], start=False, stop=True)
            coefsT = sm.tile([T, T], F32, tag="coefsT")
            nc.vector.tensor_scalar_add(coefsT[:], pc[:], mb2t_sb[:])
            coefsT_bf = sm.tile([T, T], BF16, tag="coefsTbf")
            nc.vector.tensor_copy(coefsT_bf[:], coefsT[:])

            # coefficient replication [128, 8]: cRep[(t,s), b] = 4*c[t, b]
            pr = pst.tile([128, T], F32, tag="tiny")
            nc.tensor.matmul(pr[:], iexp_sb[:], coefsT[:], start=True, stop=True)
            crep = sm.tile([128, T], F32, tag="crep")
            nc.vector.tensor_copy(crep[:], pr[:])

            # block-diagonal mixing stationary (fp8): Cb[(t,s),(b,s')] = 4c
            cb_sb = sm.tile([128, 128], E4, tag="cb")
            for b in range(BC):
                nc.vector.tensor_scalar_mul(
                    cb_sb[:, b * P:(b + 1) * P], mask16_sb[:],
                    crep[:, b:b + 1])

            # coefsB[b, t] = c[t, b]; Cdiag_t = diag(coefsB[:, t]) for layer 2
            pe2 = pst.tile([T, T], F32, tag="tiny")
            nc.tensor.matmul(pe2[:], coefsT[:], i8_sb[:], start=True, stop=True)
            coefsB = sm.tile([T, T], F32, tag="coefsB")
            nc.vector.tensor_copy(coefsB[:], pe2[:])
            i8bf = sm.tile([T, T], BF16, tag="i8bf")
            nc.vector.tensor_copy(i8bf[:], i8_sb[:])
            cdiag = sm.tile([T, T, T], BF16, tag="cdiag")   # [b', t, b]
            for t in range(T):
                nc.vector.tensor_scalar_mul(
                    cdiag[:, t, :], i8bf[:], coefsB[:, t:t + 1])

            # nb1t64[j_local, jt, b] = 64*(b1 + coefs @ db1), plus the
            # negated / x196 variants for the DVE relu-pool trick
            nb1t = sm.tile([128, KT, BC], F32, tag="nb1t")
            nb1n = sm.tile([128, KT, BC], F32, tag="nb1n")
            nb1s = sm.tile([128, KT, BC], F32, tag="nb1s")
            MULT = mybir.AluOpType.mult
            for jt in range(KT):
                pb = pst.tile([128, T], F32, tag="tiny")
                nc.tensor.matmul(pb[:], db1_sb[:, jt * 128:(jt + 1) * 128],
                                 coefsT_bf[:], start=True, stop=True)
                if jt % 2 == 0:
                    nc.vector.tensor_scalar_add(
                        nb1t[:, jt, :], pb[:], b1t_sb[:, jt:jt + 1])
                else:
                    nc.vector.tensor_scalar(
                        nb1n[:, jt, :], pb[:], b1t_sb[:, jt:jt + 1], -1.0,
                        op0=ADD, op1=MULT)
                    nc.vector.tensor_scalar(
                        nb1s[:, jt, :], pb[:], b1t_sb[:, jt:jt + 1],
                        float(NPAT), op0=ADD, op1=MULT)

            psB2.release()
            psA.release()
            pst.release()

            # ---------- phase 3: mixing (psum = 64*M) ----------
            deint_engs = (nc.gpsimd, nc.sync, nc.scalar)
            NPRE = 3        # samples whose de-interleave runs inside mixing
            mxcbs = {b: mxcp.tile([128, KT, D], E4, tag="mxcb",
                                  name=f"mxcb_pre{b}")
                     for b in range(NPRE)}
            psF = tc.alloc_tile_pool(name="psF", bufs=4, space="PSUM")
            with tc.tile_pool(name="psM", bufs=2, space="PSUM") as psM:
                for icl in range(8):
                    dwt6 = ring.tile([128, KT, D], E4, tag="dw")
                    eng = nc.sync if icl % 2 == 0 else nc.gpsimd
                    eng.dma_start(
                        dwt6[:],
                        dw1[icl * KT:(icl + 1) * KT].rearrange("k p j -> p k j"))
                    for it in range(KT):
                        pm2 = psM.tile([128, 2, 512], F32, tag="m")  # 2 banks
                        for jh in range(2):
                            nc.tensor.matmul(
                                pm2[:, jh, 0:384], cb_sb[:],
                                dwt6[:, it, jh * 384:(jh + 1) * 384],
                                start=True, stop=True)
                        # PSUM->SBUF fp8 cast split across both engines
                        # (disjoint banks -> concurrent)
                        dst = mxiall[:, icl, it, :]
                        nc.vector.tensor_copy(dst[:, 0:384], pm2[:, 0, 0:384])
                        nc.scalar.copy(dst[:, 384:768], pm2[:, 1, 0:384])
                    # de-interleave of the first NPRE samples rides along:
                    # their icl-slice only needs this icl's copies
                    for b in range(NPRE):
                        deint_engs[(icl * NPRE + b) % 3].dma_start(
                            mxcbs[b][icl * P:(icl + 1) * P, :, :],
                            mxiall[b * P:(b + 1) * P, icl, :, :])

            # ---------- phase 4: final per-sample pass ----------
            for b in range(BC):
                if b in mxcbs:
                    mxcb = mxcbs.pop(b)
                else:
                    mxcb = mxcp.tile([128, KT, D], E4, tag="mxcb")
                    for icl in range(8):
                        deint_engs[icl % 3].dma_start(
                            mxcb[icl * P:(icl + 1) * P, :, :],
                            mxiall[b * P:(b + 1) * P, icl, :, :])
                for jt in range(KT):
                    pf = psF.tile([128, NPAT], F32, tag="f")
                    # inject 64*A (= 64*X@W1), then accumulate 64*M@X
                    nc.tensor.matmul(
                        pf[:], i64_sb[:],
                        a_bf[:, jt, b * NPAT:(b + 1) * NPAT],
                        start=True, stop=False)
                    for it in range(KT):
                        nc.tensor.matmul(
                            pf[:],
                            mxcb[:, it, jt * 128:(jt + 1) * 128],
                            xt_sb[:, it, b * NPAT:(b + 1) * NPAT],
                            start=False, stop=(it == KT - 1))
                    ro = scr.tile([128, NPAT], BF16, tag="ro")
                    col = pooln[:, jt * BC + b:jt * BC + b + 1]
                    if jt % 2 == 0:
                        nc.scalar.activation(
                            ro[:], pf[:], RELU,
                            bias=nb1t[:, jt, b:b + 1], accum_out=col)
                    else:
                        nc.vector.tensor_scalar(
                            ro[:], pf[:], nb1n[:, jt, b:b + 1],
                            nb1s[:, jt, b:b + 1],
                            op0=MAX, op1=ADD, accum_out=col)

            # ---------- phase 5: layer 2 ----------
            pooln_bf = sm.tile([128, KT * BC], BF16, tag="poolnbf")
            nc.scalar.mul(pooln_bf[:], pooln[:], POOL_SCALE)
            pooln_f8 = sm.tile([128, KT * BC], E4, tag="poolnf8")
            nc.vector.tensor_scalar_mul(pooln_f8[:], pooln[:], POOL_SCALE)

            vst = sm.tile([BC, T, D], BF16, tag="vst")
            psV = tc.alloc_tile_pool(name="psV", bufs=4, space="PSUM")
            for t in range(T):
                dwt2 = ring.tile([128, KT, D], E4, tag="dw")
                eng = nc.sync if t % 2 == 0 else nc.gpsimd
                eng.dma_start(dwt2[:], dw2[t].rearrange("k p e -> p k e"))
                for eh in range(2):
                    pv = psV.tile([8, 384], F32, tag="v")
                    for kt in range(KT):
                        nc.tensor.matmul(
                            pv[:], pooln_f8[:, kt * BC:(kt + 1) * BC],
                            dwt2[:, kt, eh * 384:(eh + 1) * 384],
                            start=(kt == 0), stop=(kt == KT - 1))
                    dstv = vst[:, t, eh * 384:(eh + 1) * 384]
                    if (t * 2 + eh) % 2 == 0:
                        nc.vector.tensor_scalar_mul(dstv, pv[:], 1.0 / SD)
                    else:
                        nc.scalar.mul(dstv, pv[:], 1.0 / SD)

            out_sb = sm.tile([BC, D], F32, tag="out")
            for eh in range(2):
                po = psV.tile([8, 384], F32, tag="v")
                for kt in range(KT):
                    nc.tensor.matmul(
                        po[:], pooln_bf[:, kt * BC:(kt + 1) * BC],
                        w2_sb[:, kt, eh * 384:(eh + 1) * 384],
                        start=(kt == 0), stop=False)
                for t in range(T):
                    nc.tensor.matmul(po[:], cdiag[:, t, :],
                                     vst[:, t, eh * 384:(eh + 1) * 384],
                                     start=False, stop=False)
                nc.tensor.matmul(po[:], coefsT_bf[:],
                                 db2_sb[:, eh * 384:(eh + 1) * 384],
                                 start=False, stop=True)
                nc.vector.tensor_tensor(
                    out_sb[:, eh * 384:(eh + 1) * 384], po[:],
                    b2r_sb[:, eh * 384:(eh + 1) * 384],
                    op=ADD)
            nc.sync.dma_start(out[:], out_sb[:])
            psV.release()
            psF.release()

    _split_multi_waits(nc)
    return nc


def prep_inputs(x, W1, b1, W2, b2, dW1, db1, dW2, db2, mw1, mb1, mw2, mb2):
    """Host-side layout prep. Returns per-core in_maps."""
    bf = ml_dtypes.bfloat16
    f8 = ml_dtypes.float8_e4m3
    x = np.asarray(x); W1 = np.asarray(W1); W2 = np.asarray(W2)
    b1 = np.asarray(b1); b2 = np.asarray(b2)
    dW1 = np.asarray(dW1); dW2 = np.asarray(dW2)
    db1 = np.asarray(db1); db2 = np.asarray(db2)
    mw1 = np.asarray(mw1); mb1 = np.asarray(mb1)
    mw2 = np.asarray(mw2); mb2 = np.asarray(mb2)

    # patches^T: [B, D, NPAT]
    pt = x.reshape(B, 3, 14, P, 14, P).transpose(0, 1, 3, 5, 2, 4)
    pt = np.ascontiguousarray(pt).reshape(B, D, NPAT)

    # shared (replicated) tensors
    w1_c = np.ascontiguousarray(
        (SW * W1).reshape(KT, 128, D).transpose(1, 0, 2)).astype(f8)
    w2_c = np.ascontiguousarray(
        W2.reshape(KT, 128, D).transpose(1, 0, 2)).astype(bf)
    # dw1[icl*6+it, (t,s16), j] = 16*dW1[t, (it*8+icl)*16+s, j]
    d = (SD * dW1).reshape(T, KT, 8, P, D)      # [t, it, icl, s, j]
    dw1_c = np.ascontiguousarray(
        d.transpose(2, 1, 0, 3, 4).reshape(8 * KT, 128, D)).astype(f8)
    dw2_c = np.ascontiguousarray((SD * dW2).reshape(T, KT, 128, D)).astype(f8)
    db1_c = (SW * db1).astype(bf)
    db2_c = db2.astype(bf)
    b1t_c = np.ascontiguousarray(
        (SW * b1).reshape(KT, 128).T).astype(np.float32)
    b1tn_c = -b1t_c
    b1ts_c = NPAT * b1t_c
    b2t_c = np.ascontiguousarray(b2.reshape(KT, 128).T).astype(np.float32)
    b2r_c = np.tile(b2.astype(np.float32), (BC, 1))
    mw1_c = np.ascontiguousarray(
        mw1.reshape(KT, 128, HM).transpose(1, 0, 2)).astype(bf)
    mb1t_c = np.zeros((128, 2), np.float32)
    mb1t_c[:, 0] = mb1[:128]
    mb1t_c[:64, 1] = mb1[128:]
    mw2_c = np.zeros((128, 2, T), np.float32)
    mw2_c[:, 0, :] = mw2[:128]
    mw2_c[:64, 1, :] = mw2[128:]
    mw2_c = mw2_c.astype(bf)
    mb2t_c = mb2.reshape(T, 1).astype(np.float32)
    iexp_c = SCC * np.repeat(np.eye(T, dtype=np.float32), P, axis=1)
    i64_c = (SW * np.eye(128, dtype=np.float32)).astype(bf)
    mask16_c = np.tile(np.eye(P, dtype=np.float32), (8, 1)).astype(bf)
    i8_c = np.eye(T, dtype=np.float32)

    shared = dict(
        w1=w1_c, w2=w2_c, dw1=dw1_c, dw2=dw2_c, db1=db1_c, db2=db2_c,
        b1t=b1t_c, b1tn=b1tn_c, b1ts=b1ts_c,
        b2t=b2t_c, b2r=b2r_c, mw1=mw1_c, mb1t=mb1t_c,
        mw2=mw2_c, mb2t=mb2t_c,
        iexp=iexp_c, mask16=mask16_c, i8=i8_c, i64=i64_c,
    )

    in_maps = []
    for c in range(NCORES):
        ptc = pt[c * BC:(c + 1) * BC]                  # [BC, D, NPAT]
        # xt[p, kt, (b,n)] = ptc[b, kt*128+p, n]
        xt_c = np.ascontiguousarray(
            ptc.reshape(BC, KT, 128, NPAT).transpose(2, 1, 0, 3)
        ).reshape(128, KT, NB).astype(f8)
        m = dict(shared)
        m["xt"] = xt_c
        in_maps.append(m)
    return in_maps


_NC_CACHE = {}


def kernel(**inputs) -> np.ndarray:
    _apply_tile_patch()
    if "nc" not in _NC_CACHE:
        _NC_CACHE["nc"] = build_kernel()
    nc = _NC_CACHE["nc"]
    in_maps = prep_inputs(**inputs)
    res = run_bass_kernel_spmd(nc, in_maps, core_ids=list(range(NCORES)))
    return np.concatenate([r["out"] for r in res.results], axis=0)


# revision 35
# speedup vs baseline: 1.1046x; 1.1046x over previous
"""MetaNetImageEncoder Trainium2 kernel.

Data-parallel over batch: 8 samples per NeuronCore x 8 cores.

Per core (sample-local b in 0..7, D=768, N=196 patches, T=8 tasks):
  1. base pass:   A = P @ W1 in fp8e4m3 (W1 staged at 64x scale),
                  pooled = mean_n relu(A + b1); relu+pool alternates
                  between ScalarE (activation+accum) and VectorE
                  (tensor_scalar add/max + accum). base2 = W2.T@pooled
                  matmuls are interleaved into the phase-1 k-loop.
  2. MetaNet:     coefs[t,b] via two small matmul chains.
  3. mixing:      64*M_b = (4c)@(16*dW1) via a block-diagonal fp8
                  stationary; PSUM partitions = (sample, i%16); copies
                  cast to fp8 mxiall (64x scale).
  4. final pass:  one merged gather-DMA per sample de-interleaves
                  64*M_b; DVE adds 64*W1 (same fp8 tensor as phase 1);
                  H = relu(P @ 64*nW1 + 64*nb1), pooled_new at 64x.
  5. layer 2:     out = pooled@W2 + sum_t c (pooled@dW2[t]) + b2 + c@db2;
                  dW2 staged fp8 at 16x, un-scaled in the PSUM->SBUF copy.

All fp8 scale factors cancel: biases are staged at 64x, pooling
normalization divides by 196*64.
"""
import numpy as np
import ml_dtypes

import concourse.bass as bass
import concourse.mybir as mybir
import concourse.tile as tile
from concourse.vector_clock import ScopedClock
from concourse.bass_utils import run_bass_kernel_spmd

F32 = mybir.dt.float32
BF16 = mybir.dt.bfloat16
E4 = mybir.dt.float8e4
RELU = mybir.ActivationFunctionType.Relu
ADD = mybir.AluOpType.add
MAX = mybir.AluOpType.max

P = 16
D = 768
T = 8
HM = 192
NPAT = 196          # 14*14 patches
B = 64
NCORES = 8
BC = B // NCORES    # 8 samples per core
NB = BC * NPAT      # 1568
KT = D // 128       # 6 k-tiles

SW = 64.0           # W1 fp8 staging scale (phase-1/4 psum is 64x)
SD = 16.0           # dW1/dW2 fp8 staging scale
SCC = SW / SD       # coef scale inside cb (4.0) so cb@dw1 = 64*M
POOL_SCALE = 1.0 / (NPAT * SW)

_PATCHED = False


def _apply_tile_patch():
    """This container's walrus allows only one sem wait per instruction;
    TileContext's exit drain attaches one wait per live semaphore. Split
    them onto standalone single-wait nops."""
    global _PATCHED
    if _PATCHED:
        return
    _PATCHED = True

    def _patched(self, tick_clock, wait_clock):
        carrier = self.nc.sync.nop(nofuse=True, hint="drain_waits")
        wait_clock.add_sem_waits(
            carrier.ins, ScopedClock({None: tick_clock.global_clock})
        )
        si = carrier.ins.sync_info
        waits = list(si.on_wait) if si else []
        if len(waits) > 1:
            carrier.ins.sync_info = mybir.SyncInfo(on_wait=[waits[0]], on_update=[])
            for w in waits[1:]:
                extra = self.nc.sync.nop(nofuse=True, hint="drain_waits")
                extra.ins.sync_info = mybir.SyncInfo(on_wait=[w], on_update=[])
        self.nc.sync.drain()
        self.nc.all_engine_barrier()
        popped = self.nc._tile_sem_poison_stack.pop()
        assert popped is self._sem_poison
        self.nc.clear_and_free_semaphores(list(self.sems.allocated().values()))
        self.nc.all_engine_barrier()

    tile.TileContext._drain_and_barrier = _patched


def _split_multi_waits(nc, max_waits: int = 1):
    """Hoist extra sem waits onto same-engine InstNoOp carriers."""
    for f in nc.m.functions:
        for blk in f.blocks:
            out = []
            for inst in blk.instructions:
                si = inst.sync_info
                if si is not None and len(si.on_wait) > max_waits:
                    waits = list(si.on_wait)
                    for i, w in enumerate(waits[:-max_waits]):
                        out.append(mybir.InstNoOp(
                            name=f"{inst.name}-w{i}",
                            sync_info=mybir.SyncInfo(on_wait=[w], on_update=[]),
                            bass_nofuse=True,
                            engine=inst.engine,
                        ))
                    inst.sync_info = mybir.SyncInfo(
                        on_wait=waits[-max_waits:], on_update=list(si.on_update)
                    )
                out.append(inst)
            blk.instructions = out


def build_kernel():
    nc = bass.Bass(target_bir_lowering=False, trn_type="TRN2")

    din = {}
    def inp(name, shape, dt):
        din[name] = nc.dram_tensor(name, shape, dt, kind="ExternalInput")
        return din[name]

    xt = inp("xt", (128, KT, NB), E4)            # patches^T  [i_local, kt, (b,n)]
    w1 = inp("w1", (128, KT, D), E4)             # 64*W1 [i_local, it, j]
    w2 = inp("w2", (128, KT, D), BF16)           # W2 [j_local, jt, e]
    dw1 = inp("dw1", (48, 128, D), E4)           # 16*dW1 [icl*6+it, (t,s16), j]
    dw2 = inp("dw2", (T, KT, 128, D), E4)        # 16*dW2 [t, kt, j_local, e]
    db1 = inp("db1", (T, D), BF16)               # 64*db1
    db2 = inp("db2", (T, D), BF16)               # db2 (1x)
    b1t = inp("b1t", (128, KT), F32)             # 64*b1 [j_local, jt]
    b1tn = inp("b1tn", (128, KT), F32)           # -64*b1
    b1ts = inp("b1ts", (128, KT), F32)           # 196*64*b1
    b2t = inp("b2t", (128, KT), F32)             # b2 [e_local, et] (1x)
    b2r = inp("b2r", (BC, D), F32)               # b2 replicated over samples
    mw1 = inp("mw1", (128, KT, HM), BF16)
    mb1t = inp("mb1t", (128, 2), F32)
    mw2 = inp("mw2", (128, 2, T), BF16)          # [h_local, g, t], g=1 padded
    mb2t = inp("mb2t", (T, 1), F32)
    iexp = inp("iexp", (T, 128), F32)            # 4*repeat(eye(8),16,axis=1)
    i64 = inp("i64", (128, 128), BF16)           # 64*eye(128)
    mask16 = inp("mask16", (128, P), BF16)       # [p, s'] = (p%16==s')
    i8 = inp("i8", (T, T), F32)                  # eye(8)

    out = nc.dram_tensor("out", (BC, D), F32, kind="ExternalOutput")

    with tile.TileContext(nc) as tc:
        with (
            tc.tile_pool(name="big", bufs=1) as big,
            tc.tile_pool(name="sm", bufs=1) as sm,
            tc.tile_pool(name="ring", bufs=6) as ring,
            tc.tile_pool(name="mxcp", bufs=3) as mxcp,
            tc.tile_pool(name="scr", bufs=4) as scr,
        ):
            pst = tc.alloc_tile_pool(name="pst", bufs=2, space="PSUM")
            # ---------- persistent loads (order = queue order) ----------
            w1_sb = big.tile([128, KT, D], E4, tag="w1")
            nc.sync.dma_start(w1_sb[:, :, 0:128], w1[:, :, 0:128])
            xt_sb = big.tile([128, KT, NB], E4, tag="xt")
            for kt in range(KT):
                nc.sync.dma_start(xt_sb[:, kt, :], xt[:, kt, :])
            for jt in range(1, KT):
                nc.sync.dma_start(w1_sb[:, :, jt * 128:(jt + 1) * 128],
                                  w1[:, :, jt * 128:(jt + 1) * 128])
            w2_sb = big.tile([128, KT, D], BF16, tag="w2")
            nc.scalar.dma_start(w2_sb[:], w2[:])
            mxiall = big.tile([128, 8, KT, D], E4, tag="mxiall")
            a_bf = big.tile([128, KT, NB], BF16, tag="a_bf")
            i64_sb = sm.tile([128, 128], BF16, tag="i64")
            nc.scalar.dma_start(i64_sb[:], i64[:])

            # smalls go on the gpsimd queue: scalar must stay free for
            # phase-1 activations, sync streams xt/w1
            b1t_sb = sm.tile([128, KT], F32, tag="b1t")
            nc.gpsimd.dma_start(b1t_sb[:], b1t[:])
            b1tn_sb = sm.tile([128, KT], F32, tag="b1tn")
            nc.gpsimd.dma_start(b1tn_sb[:], b1tn[:])
            b1ts_sb = sm.tile([128, KT], F32, tag="b1ts")
            nc.gpsimd.dma_start(b1ts_sb[:], b1ts[:])
            b2t_sb = sm.tile([128, KT], F32, tag="b2t")
            nc.gpsimd.dma_start(b2t_sb[:], b2t[:])
            b2r_sb = sm.tile([BC, D], F32, tag="b2r")
            nc.gpsimd.dma_start(b2r_sb[:], b2r[:])
            mw1_sb = sm.tile([128, KT, HM], BF16, tag="mw1")
            nc.gpsimd.dma_start(mw1_sb[:], mw1[:])
            mb1t_sb = sm.tile([128, 2], F32, tag="mb1t")
            nc.gpsimd.dma_start(mb1t_sb[:], mb1t[:])
            mw2_sb = sm.tile([128, 2, T], BF16, tag="mw2")
            nc.gpsimd.dma_start(mw2_sb[:], mw2[:])
            mb2t_sb = sm.tile([T, 1], F32, tag="mb2t")
            nc.gpsimd.dma_start(mb2t_sb[:], mb2t[:])
            iexp_sb = sm.tile([T, 128], F32, tag="iexp")
            nc.gpsimd.dma_start(iexp_sb[:], iexp[:])
            mask16_sb = sm.tile([128, P], BF16, tag="mask16")
            nc.gpsimd.dma_start(mask16_sb[:], mask16[:])
            i8_sb = sm.tile([T, T], F32, tag="i8")
            nc.gpsimd.dma_start(i8_sb[:], i8[:])
            db1_sb = sm.tile([T, D], BF16, tag="db1")
            nc.gpsimd.dma_start(db1_sb[:], db1[:])
            db2_sb = sm.tile([T, D], BF16, tag="db2")
            nc.gpsimd.dma_start(db2_sb[:], db2[:])

            # pre-warm the ACT spline table so ACT_TABLE_LOAD doesn't
            # stall the first real activation mid-phase-1
            warm = scr.tile([1, 1], BF16, tag="warm")
            nc.scalar.activation(warm[:], mb1t_sb[0:1, 0:1], RELU)

            poolb = sm.tile([128, KT * BC], F32, tag="poolb")
            pooln = sm.tile([128, KT * BC], F32, tag="pooln")
            poolb_bf = sm.tile([128, KT * BC], BF16, tag="poolbbf")

            # ---------- phase 1: base pass (+ interleaved base2) ----------
            psA = tc.alloc_tile_pool(name="psA", bufs=4, space="PSUM")
            psB2 = tc.alloc_tile_pool(name="psB2", bufs=2, space="PSUM")
            b2acc = sm.tile([128, KT, BC], F32, tag="b2acc")
            for jt in range(KT):
                for ch in range(4):      # 4 chunks of 392 = 2 samples
                    pa = psA.tile([128, 392], F32, tag="a")
                    for kt in range(KT):
                        nc.tensor.matmul(
                            pa[:],
                            w1_sb[:, kt, jt * 128:(jt + 1) * 128],
                            xt_sb[:, kt, ch * 392:(ch + 1) * 392],
                            start=(kt == 0), stop=(kt == KT - 1))
                    # stash A = X@W1 (1x) for phase-4 re-injection
                    nc.vector.tensor_scalar_mul(
                        a_bf[:, jt, ch * 392:(ch + 1) * 392], pa[:],
                        1.0 / SW)
                    for bi in range(2):
                        b = ch * 2 + bi
                        ro = scr.tile([128, NPAT], BF16, tag="ro")
                        col = poolb[:, jt * BC + b:jt * BC + b + 1]
                        if b % 2 == 0:
                            nc.scalar.activation(
                                ro[:], pa[:, bi * NPAT:(bi + 1) * NPAT], RELU,
                                bias=b1t_sb[:, jt:jt + 1], accum_out=col)
                        else:
                            # relu(a+b) pooled: out = max(a,-b) (discarded),
                            # accum = sum(max(a,-b)) + 196*b
                            nc.vector.tensor_scalar(
                                ro[:], pa[:, bi * NPAT:(bi + 1) * NPAT],
                                b1tn_sb[:, jt:jt + 1], b1ts_sb[:, jt:jt + 1],
                                op0=MAX, op1=ADD, accum_out=col)
                # base2 partial for this k-block, accumulated in SBUF f32
                nc.scalar.mul(poolb_bf[:, jt * BC:(jt + 1) * BC],
                              poolb[:, jt * BC:(jt + 1) * BC], POOL_SCALE)
                pb2 = psB2.tile([128, KT, BC], F32, tag="b2p")
                for et in range(KT):
                    nc.tensor.matmul(
                        pb2[:, et, :], w2_sb[:, jt, et * 128:(et + 1) * 128],
                        poolb_bf[:, jt * BC:(jt + 1) * BC],
                        start=True, stop=True)
                if jt == 0:
                    nc.vector.tensor_copy(b2acc[:], pb2[:])
                else:
                    nc.vector.tensor_tensor(b2acc[:], b2acc[:], pb2[:], op=ADD)

            # base2^T[e, b] = W2.T @ pooled + b2  (input to MetaNet)
            base2_bf = sm.tile([128, KT * BC], BF16, tag="base2bf")
            for et in range(KT):
                nc.vector.tensor_scalar_add(
                    base2_bf[:, et * BC:(et + 1) * BC], b2acc[:, et, :],
                    b2t_sb[:, et:et + 1])

            # ---------- phase 2: MetaNet ----------
            mh0 = sm.tile([128, T], BF16, tag="mh0")
            mh1 = sm.tile([64, T], BF16, tag="mh1")
            for g, mh_g in ((0, mh0), (1, mh1)):
                cols = 128 if g == 0 else 64
                pm = pst.tile([cols, T], F32, tag="tiny")
                for kt in range(KT):
                    nc.tensor.matmul(
                        pm[:], mw1_sb[:, kt, g * 128:g * 128 + cols],
                        base2_bf[:, kt * BC:(kt + 1) * BC],
                        start=(kt == 0), stop=(kt == KT - 1))
                nc.scalar.activation(mh_g[:], pm[:], RELU,
                                     bias=mb1t_sb[:cols, g:g + 1])

            pc = pst.tile([T, T], F32, tag="tiny")
            nc.tensor.matmul(pc[:], mw2_sb[:, 0, :], mh0[:], start=True, stop=False)
            nc.tensor.matmul(pc[:], mw2_sb[0:64, 1, :], mh1[:], start=False, stop=True)
            coefsT = sm.tile([T, T], F32, tag="coefsT")
            nc.vector.tensor_scalar_add(coefsT[:], pc[:], mb2t_sb[:])
            coefsT_bf = sm.tile([T, T], BF16, tag="coefsTbf")
            nc.vector.tensor_copy(coefsT_bf[:], coefsT[:])

            # coefficient replication [128, 8]: cRep[(t,s), b] = 4*c[t, b]
            pr = pst.tile([128, T], F32, tag="tiny")
            nc.tensor.matmul(pr[:], iexp_sb[:], coefsT[:], start=True, stop=True)
            crep = sm.tile([128, T], F32, tag="crep")
            nc.vector.tensor_copy(crep[:], pr[:])

            # block-diagonal mixing stationary (fp8): Cb[(t,s),(b,s')] = 4c
            cb_sb = sm.tile([128, 128], E4, tag="cb")
            for b in range(BC):
                nc.vector.tensor_scalar_mul(
                    cb_sb[:, b * P:(b + 1) * P], mask16_sb[:],
                    crep[:, b:b + 1])

            # coefsB[b, t] = c[t, b]; Cdiag_t = diag(coefsB[:, t]) for layer 2
            pe2 = pst.tile([T, T], F32, tag="tiny")
            nc.tensor.matmul(pe2[:], coefsT[:], i8_sb[:], start=True, stop=True)
            coefsB = sm.tile([T, T], F32, tag="coefsB")
            nc.vector.tensor_copy(coefsB[:], pe2[:])
            i8bf = sm.tile([T, T], BF16, tag="i8bf")
            nc.vector.tensor_copy(i8bf[:], i8_sb[:])
            cdiag = sm.tile([T, T, T], BF16, tag="cdiag")   # [b', t, b]
            for t in range(T):
                nc.vector.tensor_scalar_mul(
                    cdiag[:, t, :], i8bf[:], coefsB[:, t:t + 1])

            # nb1t64[j_local, jt, b] = 64*(b1 + coefs @ db1), plus the
            # negated / x196 variants for the DVE relu-pool trick
            nb1t = sm.tile([128, KT, BC], F32, tag="nb1t")
            nb1n = sm.tile([128, KT, BC], F32, tag="nb1n")
            nb1s = sm.tile([128, KT, BC], F32, tag="nb1s")
            MULT = mybir.AluOpType.mult
            for jt in range(KT):
                pb = pst.tile([128, T], F32, tag="tiny")
                nc.tensor.matmul(pb[:], db1_sb[:, jt * 128:(jt + 1) * 128],
                                 coefsT_bf[:], start=True, stop=True)
                if jt % 2 == 0:
                    nc.vector.tensor_scalar_add(
                        nb1t[:, jt, :], pb[:], b1t_sb[:, jt:jt + 1])
                else:
                    nc.vector.tensor_scalar(
                        nb1n[:, jt, :], pb[:], b1t_sb[:, jt:jt + 1], -1.0,
                        op0=ADD, op1=MULT)
                    nc.vector.tensor_scalar(
                        nb1s[:, jt, :], pb[:], b1t_sb[:, jt:jt + 1],
                        float(NPAT), op0=ADD, op1=MULT)

            psB2.release()
            psA.release()
            pst.release()

            # ---------- phase 3: mixing (psum = 64*M) ----------
            deint_engs = (nc.gpsimd, nc.sync, nc.scalar)
            NPRE = 3        # samples whose de-interleave runs inside mixing
            mxcbs = {b: mxcp.tile([128, KT, D], E4, tag="mxcb",
                                  name=f"mxcb_pre{b}")
                     for b in range(NPRE)}
            psM = tc.alloc_tile_pool(name="psM", bufs=3, space="PSUM")
            if True:
                for icl in range(8):
                    dwt6 = ring.tile([128, KT, D], E4, tag="dw")
                    eng = nc.sync if icl % 2 == 0 else nc.gpsimd
                    eng.dma_start(
                        dwt6[:],
                        dw1[icl * KT:(icl + 1) * KT].rearrange("k p j -> p k j"))
                    for it in range(KT):
                        pm2 = psM.tile([128, 2, 512], F32, tag="m")  # 2 banks
                        for jh in range(2):
                            nc.tensor.matmul(
                                pm2[:, jh, 0:384], cb_sb[:],
                                dwt6[:, it, jh * 384:(jh + 1) * 384],
                                start=True, stop=True)
                        # PSUM->SBUF fp8 cast split across both engines
                        # (disjoint banks -> concurrent)
                        dst = mxiall[:, icl, it, :]
                        nc.vector.tensor_copy(dst[:, 0:384], pm2[:, 0, 0:384])
                        nc.scalar.copy(dst[:, 384:768], pm2[:, 1, 0:384])
                    # de-interleave of the first NPRE samples rides along:
                    # their icl-slice only needs this icl's copies
                    for b in range(NPRE):
                        deint_engs[(icl * NPRE + b) % 3].dma_start(
                            mxcbs[b][icl * P:(icl + 1) * P, :, :],
                            mxiall[b * P:(b + 1) * P, icl, :, :])

            # ---------- phase 4: final per-sample pass ----------
            for b in range(BC):
                if b in mxcbs:
                    mxcb = mxcbs.pop(b)
                else:
                    mxcb = mxcp.tile([128, KT, D], E4, tag="mxcb")
                    for icl in range(8):
                        deint_engs[icl % 3].dma_start(
                            mxcb[icl * P:(icl + 1) * P, :, :],
                            mxiall[b * P:(b + 1) * P, icl, :, :])
                for jt in range(KT):
                    pft = psM.tile([128, 2, 512], F32, tag="m",
                                   name=f"pf_{b}_{jt}")
                    pf = pft[:, 0, 0:NPAT]
                    # inject 64*A (= 64*X@W1), then accumulate 64*M@X
                    nc.tensor.matmul(
                        pf, i64_sb[:],
                        a_bf[:, jt, b * NPAT:(b + 1) * NPAT],
                        start=True, stop=False)
                    for it in range(KT):
                        nc.tensor.matmul(
                            pf,
                            mxcb[:, it, jt * 128:(jt + 1) * 128],
                            xt_sb[:, it, b * NPAT:(b + 1) * NPAT],
                            start=False, stop=(it == KT - 1))
                    ro = scr.tile([128, NPAT], BF16, tag="ro")
                    col = pooln[:, jt * BC + b:jt * BC + b + 1]
                    if jt % 2 == 0:
                        nc.scalar.activation(
                            ro[:], pf, RELU,
                            bias=nb1t[:, jt, b:b + 1], accum_out=col)
                    else:
                        nc.vector.tensor_scalar(
                            ro[:], pf, nb1n[:, jt, b:b + 1],
                            nb1s[:, jt, b:b + 1],
                            op0=MAX, op1=ADD, accum_out=col)

            # ---------- phase 5: layer 2 ----------
            psM.release()
            pooln_bf = sm.tile([128, KT * BC], BF16, tag="poolnbf")
            nc.scalar.mul(pooln_bf[:], pooln[:], POOL_SCALE)
            pooln_f8 = sm.tile([128, KT * BC], E4, tag="poolnf8")
            nc.vector.tensor_scalar_mul(pooln_f8[:], pooln[:], POOL_SCALE)

            vst = sm.tile([BC, T, D], BF16, tag="vst")
            psV = tc.alloc_tile_pool(name="psV", bufs=4, space="PSUM")
            for t in range(T):
                dwt2 = ring.tile([128, KT, D], E4, tag="dw")
                eng = nc.sync if t % 2 == 0 else nc.gpsimd
                eng.dma_start(dwt2[:], dw2[t].rearrange("k p e -> p k e"))
                for eh in range(2):
                    pv = psV.tile([8, 384], F32, tag="v")
                    for kt in range(KT):
                        nc.tensor.matmul(
                            pv[:], pooln_f8[:, kt * BC:(kt + 1) * BC],
                            dwt2[:, kt, eh * 384:(eh + 1) * 384],
                            start=(kt == 0), stop=(kt == KT - 1))
                    dstv = vst[:, t, eh * 384:(eh + 1) * 384]
                    if (t * 2 + eh) % 2 == 0:
                        nc.vector.tensor_scalar_mul(dstv, pv[:], 1.0 / SD)
                    else:
                        nc.scalar.mul(dstv, pv[:], 1.0 / SD)

            out_sb = sm.tile([BC, D], F32, tag="out")
            for eh in range(2):
                po = psV.tile([8, 384], F32, tag="v")
                for kt in range(KT):
                    nc.tensor.matmul(
                        po[:], pooln_bf[:, kt * BC:(kt + 1) * BC],
                        w2_sb[:, kt, eh * 384:(eh + 1) * 384],
                        start=(kt == 0), stop=False)
                for t in range(T):
                    nc.tensor.matmul(po[:], cdiag[:, t, :],
                                     vst[:, t, eh * 384:(eh + 1) * 384],
                                     start=False, stop=False)
                nc.tensor.matmul(po[:], coefsT_bf[:],
                                 db2_sb[:, eh * 384:(eh + 1) * 384],
                                 start=False, stop=True)
                nc.vector.tensor_tensor(
                    out_sb[:, eh * 384:(eh + 1) * 384], po[:],
                    b2r_sb[:, eh * 384:(eh + 1) * 384],
                    op=ADD)
            nc.sync.dma_start(out[:], out_sb[:])
            psV.release()

    _split_multi_waits(nc)
    return nc


def prep_inputs(x, W1, b1, W2, b2, dW1, db1, dW2, db2, mw1, mb1, mw2, mb2):
    """Host-side layout prep. Returns per-core in_maps."""
    bf = ml_dtypes.bfloat16
    f8 = ml_dtypes.float8_e4m3
    x = np.asarray(x); W1 = np.asarray(W1); W2 = np.asarray(W2)
    b1 = np.asarray(b1); b2 = np.asarray(b2)
    dW1 = np.asarray(dW1); dW2 = np.asarray(dW2)
    db1 = np.asarray(db1); db2 = np.asarray(db2)
    mw1 = np.asarray(mw1); mb1 = np.asarray(mb1)
    mw2 = np.asarray(mw2); mb2 = np.asarray(mb2)

    # patches^T: [B, D, NPAT]
    pt = x.reshape(B, 3, 14, P, 14, P).transpose(0, 1, 3, 5, 2, 4)
    pt = np.ascontiguousarray(pt).reshape(B, D, NPAT)

    # shared (replicated) tensors
    w1_c = np.ascontiguousarray(
        (SW * W1).reshape(KT, 128, D).transpose(1, 0, 2)).astype(f8)
    w2_c = np.ascontiguousarray(
        W2.reshape(KT, 128, D).transpose(1, 0, 2)).astype(bf)
    # dw1[icl*6+it, (t,s16), j] = 16*dW1[t, (it*8+icl)*16+s, j]
    d = (SD * dW1).reshape(T, KT, 8, P, D)      # [t, it, icl, s, j]
    dw1_c = np.ascontiguousarray(
        d.transpose(2, 1, 0, 3, 4).reshape(8 * KT, 128, D)).astype(f8)
    dw2_c = np.ascontiguousarray((SD * dW2).reshape(T, KT, 128, D)).astype(f8)
    db1_c = (SW * db1).astype(bf)
    db2_c = db2.astype(bf)
    b1t_c = np.ascontiguousarray(
        (SW * b1).reshape(KT, 128).T).astype(np.float32)
    b1tn_c = -b1t_c
    b1ts_c = NPAT * b1t_c
    b2t_c = np.ascontiguousarray(b2.reshape(KT, 128).T).astype(np.float32)
    b2r_c = np.tile(b2.astype(np.float32), (BC, 1))
    mw1_c = np.ascontiguousarray(
        mw1.reshape(KT, 128, HM).transpose(1, 0, 2)).astype(bf)
    mb1t_c = np.zeros((128, 2), np.float32)
    mb1t_c[:, 0] = mb1[:128]
    mb1t_c[:64, 1] = mb1[128:]
    mw2_c = np.zeros((128, 2, T), np.float32)
    mw2_c[:, 0, :] = mw2[:128]
    mw2_c[:64, 1, :] = mw2[128:]
    mw2_c = mw2_c.astype(bf)
    mb2t_c = mb2.reshape(T, 1).astype(np.float32)
    iexp_c = SCC * np.repeat(np.eye(T, dtype=np.float32), P, axis=1)
    i64_c = (SW * np.eye(128, dtype=np.float32)).astype(bf)
    mask16_c = np.tile(np.eye(P, dtype=np.float32), (8, 1)).astype(bf)
    i8_c = np.eye(T, dtype=np.float32)

    shared = dict(
        w1=w1_c, w2=w2_c, dw1=dw1_c, dw2=dw2_c, db1=db1_c, db2=db2_c,
        b1t=b1t_c, b1tn=b1tn_c, b1ts=b1ts_c,
        b2t=b2t_c, b2r=b2r_c, mw1=mw1_c, mb1t=mb1t_c,
        mw2=mw2_c, mb2t=mb2t_c,
        iexp=iexp_c, mask16=mask16_c, i8=i8_c, i64=i64_c,
    )

    in_maps = []
    for c in range(NCORES):
        ptc = pt[c * BC:(c + 1) * BC]                  # [BC, D, NPAT]
        # xt[p, kt, (b,n)] = ptc[b, kt*128+p, n]
        xt_c = np.ascontiguousarray(
            ptc.reshape(BC, KT, 128, NPAT).transpose(2, 1, 0, 3)
        ).reshape(128, KT, NB).astype(f8)
        m = dict(shared)
        m["xt"] = xt_c
        in_maps.append(m)
    return in_maps


_NC_CACHE = {}


def kernel(**inputs) -> np.ndarray:
    _apply_tile_patch()
    if "nc" not in _NC_CACHE:
        _NC_CACHE["nc"] = build_kernel()
    nc = _NC_CACHE["nc"]
    in_maps = prep_inputs(**inputs)
    res = run_bass_kernel_spmd(nc, in_maps, core_ids=list(range(NCORES)))
    return np.concatenate([r["out"] for r in res.results], axis=0)
